# revision 1
# baseline (speedup 1.0000x reference)
"""Ragged grouped-GEMM (MoE group linear) on 8 trn2 NeuronCores.

y[s_g:e_g] = x[s_g:e_g] @ w[g].T  for 64 expert segments given by
cumulative offsets.

Strategy: token-shard 8192 tokens per core (perfectly balanced
compute).  Matmuls are weight-stationary: lhsT = a [128k x 128dout]
tile of the expert weight, rhs = x^T [128k x N tokens], out =
[128dout x N] in PSUM.  Matmul cost on the PE is proportional to the
MOVING (token) dim only, so ragged segments cost exactly their token
count -- no 128-token tile padding.  The host packs each core's
per-expert token segments into a shared static "slot" inventory
(token-granular capacities, one 2MB weight DMA per slot, tokens
processed in <=512-token PSUM chunks).  The slot inventory is
optimized for the actual offsets at compile time by a local search
and shared by all 8 cores, so a single static SPMD program serves
all cores; per-core raggedness lives entirely in the data (which
expert weight / which tokens each slot carries).  Matmuls run in
bf16 with fp32 PSUM accumulation; x is pre-transposed on the host
(feature-major per chunk) and y is written back transposed in bf16
and re-assembled on the host.
"""

import bisect
import math
import os
import time
import numpy as np
import ml_dtypes

import concourse.bass as bass
import concourse.mybir as mybir
import concourse.tile as tile
from concourse import bacc
from concourse.bass_utils import run_bass_kernel_spmd

T_TOK = 65536
G_EXP = 64
DIN = 1024
DOUT = 1024
NCORES = 8
TPC = T_TOK // NCORES
KTILES = DIN // 128
JTILES = DOUT // 128
CHUNK = 512  # PSUM bank: 512 fp32 per partition
BF16 = mybir.dt.bfloat16
F32 = mybir.dt.float32

_COMPILED = {}
LAST_EXEC_NS = None
SPMD_WALL_S = None


def _segments(offs):
    """Per-core list of (expert, tok_start, ntokens)."""
    bounds = np.concatenate([[0], np.asarray(offs, dtype=np.int64)])
    per_core = []
    for c in range(NCORES):
        lo, hi = c * TPC, (c + 1) * TPC
        segs = []
        for g in range(G_EXP):
            s, e = max(int(bounds[g]), lo), min(int(bounds[g + 1]), hi)
            if e > s:
                segs.append((g, s, e - s))
        per_core.append(segs)
    return per_core


def _greedy_split(free, n):
    """Cover n with slots from free (list of (cap, idx) asc): repeatedly
    smallest >= rem, else largest.  Returns combo list or None."""
    rest = list(free)
    combo = []
    rem = n
    while rem > 0:
        if not rest:
            return None
        caps = [c for c, _ in rest]
        j = bisect.bisect_left(caps, rem)
        if j < len(rest):
            combo.append(rest.pop(j))
            rem = 0
        else:
            c, i = rest.pop()
            combo.append((c, i))
            rem -= c
    return combo


def _fit_core(sizes, inv, budget=120000):
    """Branch&bound fit of segment sizes onto slot capacities.  Per
    segment (desc) branch over: smallest single fitting slot, greedy
    multi-slot split, min-waste pair.  Returns list of
    (seg_i, [(slot_i, used), ...]) or None."""
    order = sorted(range(len(sizes)), key=lambda i: -sizes[i])
    free0 = tuple(sorted((c, i) for i, c in enumerate(inv)))
    seen = set()
    count = [0]

    def rec(si, free):
        if si == len(order):
            return []
        key = (si, tuple(c for c, _ in free))
        if key in seen:
            return None
        count[0] += 1
        if count[0] > budget:
            return None
        n = sizes[order[si]]
        caps = [c for c, _ in free]
        opts = []
        j = bisect.bisect_left(caps, n)
        if j < len(free):
            opts.append((free[j],))
        if free and caps[-1] < n:
            g = _greedy_split(free, n)
            if g is not None:
                opts.append(tuple(g))
        best_pair = None
        for a_i in range(len(free)):
            a = caps[a_i]
            if a >= n:
                break
            b_i = bisect.bisect_left(caps, n - a, a_i + 1)
            if b_i < len(free):
                wst = a + caps[b_i] - n
                if best_pair is None or wst < best_pair[0]:
                    best_pair = (wst, (free[a_i], free[b_i]))
        if best_pair is not None:
            opts.append(best_pair[1])
        for combo in opts[:3]:
            taken = set(id_ for _, id_ in combo)
            nf = tuple(f for f in free if f[1] not in taken)
            sub = rec(si + 1, nf)
            if sub is not None:
                rem = n
                grp = []
                for c, i in sorted(combo, reverse=True):
                    used = min(c, rem)
                    grp.append((i, used))
                    rem -= used
                return [(order[si], grp)] + sub
        seen.add(key)
        return None

    return rec(0, free0)


def _inv_cost(inv, max_slots=26):
    if len(inv) > max_slots or not inv:
        return 1e18
    cap = sum(inv)
    nch = sum((c + CHUNK - 1) // CHUNK for c in inv)
    pe = 26.67 * cap + 141 * nch
    pen = max(0.0, 6316.0 * len(inv) - 0.8 * pe) * 2
    return pe + pen


def _search_inventory(profiles, restarts=10, iters=12000):
    """Local search for a shared slot-capacity multiset minimizing the
    PE cost model, feasible for every core's segment-size profile.
    Fully deterministic (fixed iteration counts, fixed seed)."""
    def feasible(inv):
        return all(_fit_core(p, inv) is not None for p in profiles)

    split = []
    for segs in profiles:
        ss = []
        for n in segs:
            while n > 2048:
                ss.append(2048)
                n -= 2048
            ss.append(n)
        split.append(sorted(ss, reverse=True))
    m = max(len(s) for s in split)
    inv = [max(s[i] if i < len(s) else 0 for s in split) for i in range(m)]
    inv = [c for c in inv if c > 0]
    if not feasible(inv):  # paranoia: grow until feasible
        while not feasible(inv):
            inv = inv + [2048]
    best, bcost = list(inv), _inv_cost(inv)
    for seed in range(restarts):
        rng = np.random.default_rng(seed)
        cur, ccost = list(best), bcost
        for it in range(iters):
            cand = list(cur)
            move = rng.integers(0, 5)
            if move == 0 and cand:
                i = int(rng.integers(0, len(cand)))
                cand[i] -= int(rng.choice([1, 2, 4, 8, 16, 32, 64, 128, 256]))
                if cand[i] <= 0:
                    cand.pop(i)
            elif move == 1 and cand:
                cand.pop(int(rng.integers(0, len(cand))))
            elif move == 2 and cand:
                i = int(rng.integers(0, len(cand)))
                if cand[i] >= 2:
                    a = int(rng.integers(1, cand[i]))
                    cand.append(cand[i] - a)
                    cand[i] = a
            elif move == 3 and len(cand) >= 2:
                i, j = rng.choice(len(cand), 2, replace=False)
                cand[int(i)] += cand[int(j)]
                cand.pop(int(j))
            elif cand:
                i = int(rng.integers(0, len(cand)))
                cand[i] += int(rng.choice([1, 2, 4, 8, 16, 32]))
            cand = [c for c in cand if c > 0]
            if not cand or not feasible(cand):
                continue
            cc = _inv_cost(cand)
            if cc <= ccost + (30 if it % 3 else 0):
                cur, ccost = cand, cc
                if cc < bcost:
                    best, bcost = list(cand), cc

    # deterministic compaction: shrink each slot to its min feasible size
    # (binary search, holding the others fixed), then drop empty slots;
    # repeat until a full pass makes no progress.
    improved = True
    while improved:
        improved = False
        for i in range(len(best)):
            lo, hi = 0, best[i]
            while lo < hi:
                mid = (lo + hi) // 2
                trial = best[:i] + ([mid] if mid else []) + best[i + 1:]
                if feasible(trial):
                    hi = mid
                else:
                    lo = mid + 1
            if lo < best[i]:
                best[i] = lo
                improved = True
        best = [c for c in best if c > 0]
    return sorted(best, reverse=True)


WBUFS = 6
XBUFS = 3


def _mini_sim(caps, wbufs=WBUFS, xbufs=XBUFS):
    """Approximate makespan model: SP ring (w + x transfers, FIFO, gated
    by tile-pool buffer depth) feeding the PE (64 matmuls per chunk).
    Used to pick the slot ORDER -- tiny slots need a full 2MB weight for
    little PE time, so they must hide behind big slots' PE runway."""
    WCOST = 16384 * 0.3855
    SEM = 900.0
    ring_t = 1215.0          # first dma issue + DGE delay
    pe_t = 601.0 + 31 * 107  # warmup dummies end
    chunk_end = []
    slot_end = []
    # slot 0's weight + first x chunk are k-interleaved on the ring, so
    # the k-outer first chunk starts after ~one (w,x) slice pair
    w_avail_hd = ring_t + WCOST / KTILES + SEM
    for s, cap in enumerate(caps):
        if s >= wbufs - 1 and len(slot_end) >= wbufs - 1:
            ring_t = max(ring_t, slot_end[s - (wbufs - 1)])
        ring_t += WCOST
        w_avail = (w_avail_hd if s == 0 else ring_t + SEM)
        for ci, n in enumerate(_chunks_of(cap)):
            g = len(chunk_end)
            if g >= xbufs:
                ring_t = max(ring_t, chunk_end[g - xbufs])
            ring_t += 16 * n * 0.3855
            x_avail = ring_t + SEM
            if s == 0 and ci == 0:
                x_avail = w_avail_hd + 16 * n * 0.3855 / KTILES
            start = max(pe_t, w_avail, x_avail)
            pe_t = start + 64 * (n * 0.4166 + 2.2)
            chunk_end.append(pe_t)
        slot_end.append(pe_t)
    n_last = _chunks_of(caps[-1])[-1]
    return pe_t + 1150 + 8 * 2 * n_last * 0.3855 + SEM


def _order_slots(slot_caps):
    """Deterministic slot-order search: start from big/small interleave,
    hill-climb pairwise swaps on the mini-sim makespan."""
    desc = sorted(range(len(slot_caps)), key=lambda i: -slot_caps[i])
    lo, hi = 0, len(desc) - 1
    order = []
    while lo <= hi:
        order.append(desc[lo])
        lo += 1
        if lo <= hi:
            order.append(desc[hi])
            hi -= 1
    best = order
    bcost = _mini_sim([slot_caps[i] for i in best])
    improved = True
    while improved:
        improved = False
        for a in range(len(best)):
            for b in range(a + 1, len(best)):
                cand = list(best)
                cand[a], cand[b] = cand[b], cand[a]
                cc = _mini_sim([slot_caps[i] for i in cand])
                if cc < bcost - 1e-9:
                    best, bcost = cand, cc
                    improved = True
    return best


_PLANS = {}

# Precomputed inventories from long offline anneals (tuned for the
# expected offset instance).  Used only when feasible for the actual
# offsets -- otherwise the deterministic runtime search runs.
_CANDIDATE_INVS = [
    [1907, 1423, 1119, 770, 504, 456, 402, 352, 292, 250, 233, 218,
     151, 136, 100, 41, 21],
]


def _plan(offs):
    """Returns (slot_caps, fills): shared slot capacities (tokens, desc)
    and per-core slot fills [(expert, tok_start, n_used) or None].
    Cached by offsets so repeated calls agree."""
    pkey = np.asarray(offs, dtype=np.int64).tobytes()
    if pkey in _PLANS:
        return _PLANS[pkey]
    per_core = _segments(offs)
    profiles = [[n for _, _, n in segs] for segs in per_core]
    slot_caps = None
    cands = [c for c in _CANDIDATE_INVS
             if all(_fit_core(p, c, budget=250000) is not None
                    for p in profiles)]
    if cands:
        slot_caps = sorted(min(cands, key=_inv_cost), reverse=True)
    else:
        slot_caps = _search_inventory(profiles)
    order = _order_slots(slot_caps)
    slot_caps = [slot_caps[i] for i in order]

    fills = []
    for c in range(NCORES):
        segs = per_core[c]
        fit = _fit_core([n for _, _, n in segs], slot_caps, budget=250000)
        fill = [None] * len(slot_caps)
        for si, grp in fit:
            g, s, _ = segs[si]
            base = s
            for slot_i, used in grp:
                fill[slot_i] = (g, base, used)
                base += used
        fills.append(fill)
    _PLANS[pkey] = (slot_caps, fills)
    return slot_caps, fills


def _chunks_of(cap):
    out = [CHUNK] * (cap // CHUNK)
    if cap % CHUNK:
        out.append(cap % CHUNK)
    return out


def _build_inputs(x, w, slot_caps, fills):
    """Host-side pack: xt [128, 8*cap] bf16 (feature-major per chunk),
    wt [nslots, 128, 8192] bf16, token index array per core."""
    cap_tot = sum(slot_caps)
    slot_off = np.concatenate([[0], np.cumsum(slot_caps)])
    wt_cache = {}

    def wt_of(g):
        if g not in wt_cache:
            wtg = w[g].T.astype(ml_dtypes.bfloat16)
            wt_cache[g] = np.ascontiguousarray(
                wtg.reshape(KTILES, 128, DOUT).transpose(1, 0, 2)
            ).reshape(128, KTILES * DOUT)
        return wt_cache[g]

    # chunk table (shared): (tok_off, n)
    chunks = []
    for s, cap in enumerate(slot_caps):
        o = int(slot_off[s])
        for n in _chunks_of(cap):
            chunks.append((o, n))
            o += n

    xts, wts, idxs = [], [], []
    for c in range(NCORES):
        idx = np.full(cap_tot, -1, dtype=np.int64)
        wt_c = np.zeros((len(slot_caps), 128, KTILES * DOUT),
                        dtype=ml_dtypes.bfloat16)
        for s, piece in enumerate(fills[c]):
            if piece is None:
                continue
            g, base, used = piece
            wt_c[s] = wt_of(g)
            o = int(slot_off[s])
            idx[o:o + used] = np.arange(base, base + used)
        xpad = np.zeros((cap_tot, DIN), dtype=ml_dtypes.bfloat16)
        valid = idx >= 0
        xpad[valid] = x[idx[valid]].astype(ml_dtypes.bfloat16)
        xt_c = np.empty((128, KTILES * cap_tot), dtype=ml_dtypes.bfloat16)
        for o, n in chunks:
            xt_c[:, KTILES * o:KTILES * (o + n)] = (
                xpad[o:o + n].reshape(n, KTILES, 128)
                .transpose(2, 1, 0).reshape(128, KTILES * n)
            )
        xts.append(xt_c)
        wts.append(wt_c)
        idxs.append(idx)
    return xts, wts, idxs, cap_tot


def _build_program(key):
    slot_caps, repeat = key
    nslots = len(slot_caps)
    cap_tot = sum(slot_caps)
    stream = KTILES * cap_tot

    nc = bacc.Bacc("TRN2", target_bir_lowering=False)
    xt = nc.dram_tensor("xt", [128, stream], BF16, kind="ExternalInput")
    wt = nc.dram_tensor(
        "wt", [nslots, 128, KTILES * DOUT], BF16, kind="ExternalInput")
    y = nc.dram_tensor("y", [128, stream], BF16, kind="ExternalOutput")

    with tile.TileContext(nc) as tc:
        with (
            tc.tile_pool(name="wp", bufs=WBUFS) as wp,
            tc.tile_pool(name="xp", bufs=XBUFS) as xp,
            tc.tile_pool(name="pp", bufs=8, space="PSUM") as pp,
            tc.tile_pool(name="yp", bufs=3) as yp,
        ):
            # PE p-state warmup: the cost model runs the tensor engine at
            # half speed for the first 3us of a continuous-busy streak.
            # Burn the streak on dependency-free dummy matmuls while the
            # first weight/x DMAs are in flight so real matmuls run at
            # full clock from the start.
            dummy = xp.tile([128, 128], BF16, tag="warm", name="warm")
            nc.vector.memset(dummy[:], 0)
            for _ in range(31):
                pw = pp.tile([128, CHUNK], F32, tag="ps", name="pw")
                nc.tensor.matmul(
                    pw[:1, :128], lhsT=dummy[:, :1], rhs=dummy[:],
                    start=True, stop=True)
            for r in range(repeat):
                xoff = 0
                for s, cap in enumerate(slot_caps):
                    wtile = wp.tile([128, KTILES * DOUT], BF16, tag="w")
                    chunk_list = _chunks_of(cap)
                    head = (s == 0 and r == 0)
                    if head:
                        # first slot: weight and first x chunk DMA'd in
                        # interleaved k-slices, and the first chunk's
                        # matmuls k-outer, so the PE starts after one
                        # (w,x) k-slice pair instead of the whole 2MB
                        # weight + whole x chunk.
                        n0 = chunk_list[0]
                        xtile0 = xp.tile([128, KTILES * n0], BF16, tag="x")
                        for k in range(KTILES):
                            nc.sync.dma_start(
                                out=wtile[:, k * DOUT:(k + 1) * DOUT],
                                in_=wt[s][:, k * DOUT:(k + 1) * DOUT])
                            nc.sync.dma_start(
                                out=xtile0[:, k * n0:(k + 1) * n0],
                                in_=xt[:, xoff + k * n0:xoff + (k + 1) * n0])
                    else:
                        half = KTILES * DOUT // 2
                        nc.sync.dma_start(
                            out=wtile[:, :half], in_=wt[s][:, :half])
                        nc.sync.dma_start(
                            out=wtile[:, half:], in_=wt[s][:, half:])
                    for ci, n in enumerate(chunk_list):
                        if head and ci == 0:
                            xtile = xtile0
                        else:
                            xtile = xp.tile([128, KTILES * n], BF16, tag="x")
                            nc.sync.dma_start(
                                out=xtile[:],
                                in_=xt[:, xoff:xoff + KTILES * n])
                        ytile = yp.tile([128, JTILES * n], BF16, tag="y")
                        if head and ci == 0:
                            # k-outer: each k pass needs only w k-slice k
                            pss = [pp.tile([128, CHUNK], F32, tag="ps",
                                           name="ps")
                                   for _ in range(JTILES)]
                            for k in range(KTILES):
                                for j in range(JTILES):
                                    nc.tensor.matmul(
                                        pss[j][:, :n],
                                        lhsT=wtile[:, k * DOUT + j * 128:
                                                   k * DOUT + (j + 1) * 128],
                                        rhs=xtile[:, k * n:(k + 1) * n],
                                        start=(k == 0), stop=(k == KTILES - 1))
                            for j in range(JTILES):
                                nc.vector.tensor_copy(
                                    ytile[:, j * n:(j + 1) * n],
                                    pss[j][:, :n])
                        else:
                            for j in range(JTILES):
                                ps = pp.tile([128, CHUNK], F32, tag="ps")
                                for k in range(KTILES):
                                    nc.tensor.matmul(
                                        ps[:, :n],
                                        lhsT=wtile[:, k * DOUT + j * 128:
                                                   k * DOUT + (j + 1) * 128],
                                        rhs=xtile[:, k * n:(k + 1) * n],
                                        start=(k == 0), stop=(k == KTILES - 1))
                                nc.vector.tensor_copy(
                                    ytile[:, j * n:(j + 1) * n], ps[:, :n])
                        last = (s == len(slot_caps) - 1
                                and ci == len(chunk_list) - 1)
                        if last:
                            # split so the first half's transfer overlaps
                            # the second half's evictions at the very tail
                            h = JTILES * n // 2
                            nc.scalar.dma_start(
                                out=y[:, xoff:xoff + h], in_=ytile[:, :h])
                            nc.scalar.dma_start(
                                out=y[:, xoff + h:xoff + JTILES * n],
                                in_=ytile[:, h:])
                        else:
                            nc.scalar.dma_start(
                                out=y[:, xoff:xoff + JTILES * n],
                                in_=ytile[:])
                        xoff += KTILES * n
    nc.compile()
    return nc


def kernel(input, weight, grouped_mm_offs):
    global LAST_EXEC_NS, SPMD_WALL_S
    x = np.ascontiguousarray(np.asarray(input, dtype=np.float32))
    w = np.ascontiguousarray(np.asarray(weight, dtype=np.float32))
    offs = np.asarray(grouped_mm_offs, dtype=np.int32)

    repeat = int(os.environ.get("KERNEL_REPEAT", "1"))
    slot_caps, fills = _plan(offs)
    key = (tuple(slot_caps), repeat)
    if key not in _COMPILED:
        _COMPILED[key] = _build_program(key)
    nc = _COMPILED[key]

    xts, wts, idxs, cap_tot = _build_inputs(x, w, slot_caps, fills)
    in_maps = [{"xt": xts[c], "wt": wts[c]} for c in range(NCORES)]
    t0 = time.time()
    res = run_bass_kernel_spmd(nc, in_maps, core_ids=list(range(NCORES)))
    SPMD_WALL_S = time.time() - t0
    LAST_EXEC_NS = res.exec_time_ns

    # decode: y[p, 8*o + j*n + t] = out[tok(o+t), j*128 + p]
    slot_off = np.concatenate([[0], np.cumsum(slot_caps)])
    chunks = []
    for s, cap in enumerate(slot_caps):
        o = int(slot_off[s])
        for n in _chunks_of(cap):
            chunks.append((o, n))
            o += n

    out = np.zeros((T_TOK, DOUT), dtype=np.float32)
    for c in range(NCORES):
        yb = np.asarray(res.results[c]["y"])
        rows = np.empty((cap_tot, DOUT), dtype=np.float32)
        for o, n in chunks:
            blk = yb[:, KTILES * o:KTILES * (o + n)].reshape(128, JTILES, n)
            rows[o:o + n] = (
                blk.transpose(2, 1, 0).reshape(n, DOUT).astype(np.float32))
        valid = idxs[c] >= 0
        out[idxs[c][valid]] = rows[valid]
    return out



# revision 4
# speedup vs baseline: 1.0325x; 1.0325x over previous
"""Ragged grouped-GEMM (MoE group linear) on 8 trn2 NeuronCores.

y[s_g:e_g] = x[s_g:e_g] @ w[g].T for 64 expert segments given by
cumulative offsets.

Strategy: the PE cost is proportional to the token (moving) dim only,
so the kernel is PE-bound at ~26.6ns/token/core.  All that matters is
(a) per-core stream length = sum of slot capacities, (b) head latency,
(c) tail drain.  Tokens are routed freely across cores (host-side
gather/scatter), so the 8 cores share ONE static SPMD program whose
slot-capacity inventory is globally assigned: cells = 8 copies of the
inventory; each cell holds one contiguous piece of one expert block
(bounded-coin exact cover, found by DP).  This reaches ~2 tokens/core
of padding vs 8192 ideal.  DMA queues are split: expert weights stream
on the gpsimd (Pool) queue, x on sync (SP), slot-0 weights + y
writeback on scalar (Activation), PSUM->SBUF copies on vector (DVE),
so the PE never waits on a shared ring.  Matmuls run in bf16 with fp32
PSUM accumulation; PE p-state is warmed by dummy matmuls sized so real
work starts the moment the first weight/x k-slices land.
"""

import time
from collections import Counter

import numpy as np
import ml_dtypes

import concourse.mybir as mybir
import concourse.tile as tile
from concourse import bacc
from concourse.bass_utils import run_bass_kernel_spmd

T_TOK = 65536
G_EXP = 64
DIN = 1024
DOUT = 1024
NCORES = 8
KTILES = DIN // 128
JTILES = DOUT // 128
CHUNK = 512  # PSUM bank: 512 fp32 per partition
BF16 = mybir.dt.bfloat16
F32 = mybir.dt.float32

# Slot-capacity inventory (per-core, process order).  Found offline by
# annealing on the global cell-cover problem for the expected offsets;
# sum = 8194 -> 2 tokens/core padding.  Verified feasible at runtime
# against the actual offsets (deterministic re-fit); falls back to a
# guaranteed-feasible construction if the offsets differ.
CAPS = [1037, 908, 878, 715, 597, 530, 484, 422, 396, 348, 318, 222,
        218, 210, 164, 145, 130, 110, 97, 76, 47, 44, 40, 36, 15, 7]

N_WARM = 22          # PE p-state warmup dummy matmuls
HEAD_CHUNK = 256     # first chunk of slot 0 (k-interleaved head)
TAIL_CHUNK = 16      # last chunk of the final slot (fast drain)
WBUFS = 8
XBUFS = 4
YBUFS = 3


WDMA = 6316.0    # 2MB weight transfer on one queue
SEM = 900.0
PE_NS = 26.61    # per token (64 matmul rows)


def _order_sim(caps):
    """Queue-pacing mini-model: slot 0's weight on scalar (free), slots
    1.. on gpsimd (serialized, WBUFS-deep pool admission).  Returns
    estimated stall-inclusive PE finish."""
    m = len(caps)
    pe_end = 3000.0 + caps[0] * PE_NS      # head start + slot 0
    ends = [pe_end]
    q_t = 1883.0
    for p in range(1, m):
        if p >= WBUFS:
            q_t = max(q_t, ends[p - WBUFS])
        q_t += WDMA
        start = max(pe_end, q_t + SEM - caps[p] * 0)
        # weight must be there before slot's first matmul
        start = max(pe_end, q_t + SEM)
        if start > pe_end:
            pe_end = start
        pe_end += caps[p] * PE_NS
        ends.append(pe_end)
    return pe_end


def _weave(caps):
    """Process order: interleave big and small slots so every slot's 2MB
    weight DMA hides under a big slot's PE runway; keeps tiny slots
    separated (their PSUM copies are fixed-cost) and ends on a mid-size
    slot for a small tail chunk."""
    desc = sorted(caps, reverse=True)
    out = []
    i, j = 0, len(desc) - 1
    while i <= j:
        out.append(desc[i])
        i += 1
        if i <= j:
            out.append(desc[j])
            j -= 1
    return out

_COMPILED = {}
_PLANS = {}
LAST_EXEC_NS = None
SPMD_WALL_S = None


# --------------------------------------------------------------- plan

def _cover_block(s, avail, vals, max_over):
    """Min reachable >= s from bounded multiset avail; prefers large
    values.  Returns (tgt, [(val, count)]) or None."""
    LIM = s + max_over + 1
    dp = np.zeros(LIM, dtype=bool)
    dp[0] = True
    stages = []
    for v in vals:
        cnt = avail[v]
        if cnt == 0 or v >= LIM:
            stages.append(None)
            continue
        stages.append(dp.copy())
        k = 1
        rem = cnt
        while rem > 0:
            take = min(k, rem)
            shift = v * take
            if shift < LIM:
                dp[shift:] |= dp[:-shift]
            rem -= take
            k <<= 1
    nz = np.nonzero(dp[s:])[0]
    if len(nz) == 0:
        return None
    tgt = s + int(nz[0])
    cur = tgt
    pieces = []
    for vi in range(len(vals) - 1, -1, -1):
        v = vals[vi]
        st = stages[vi]
        if st is None:
            continue
        take = -1
        for used in range(min(avail[v], cur // v), -1, -1):
            if st[cur - used * v]:
                take = used
                break
        if take < 0:
            return None
        if take:
            cur -= v * take
            pieces.append((v, take))
    if cur != 0:
        return None
    return tgt, pieces


def _fit_global(blocks, caps, tries=16):
    """Cover every block with cells (8 copies of caps).  Returns per-
    block piece list [(cap, used), ...] or None."""
    budget = NCORES * sum(caps) - sum(blocks)
    if budget < 0:
        return None
    n = len(blocks)
    for t in range(tries):
        if t == 0:
            order = sorted(range(n), key=lambda i: blocks[i])
        elif t == 1:
            order = sorted(range(n), key=lambda i: -blocks[i])
        else:
            rng = np.random.default_rng(t)
            order = sorted(range(n), key=lambda i: blocks[i])
            k = int(rng.integers(1, n))
            order = order[:k] + list(rng.permutation(order[k:]))
        avail = Counter()
        for c in caps:
            avail[c] += NCORES
        vals = sorted(set(caps))
        asg = [None] * n
        over_left = budget
        ok = True
        for bi in order:
            r = _cover_block(blocks[bi], avail, vals, over_left)
            if r is None:
                ok = False
                break
            tgt, pieces = r
            over_left -= tgt - blocks[bi]
            for v, k in pieces:
                avail[v] -= k
            cells = []
            for v, k in pieces:
                cells += [v] * k
            cells.sort()
            over = tgt - blocks[bi]
            out = []
            for v in cells:
                use = v
                if over > 0:
                    red = min(over, v)
                    use = v - red
                    over -= red
                out.append((v, use))
            asg[bi] = out
        if ok:
            return asg
    return None


def _fallback_fit(blocks):
    """Guaranteed-feasible plan for arbitrary offsets: consecutive desc
    groups, each slot serving only its own group's blocks (min cap with
    <= 8 cells).  Returns (caps, asg) in _fit_global's format."""
    order = sorted(range(len(blocks)), key=lambda i: -blocks[i])
    bs = [blocks[i] for i in order]
    N = len(bs)

    def min_cap(group):
        s = sum(group)
        c = max((s + 7) // 8, 1)
        while sum((b + c - 1) // c for b in group) > 8:
            c += 1
        return c

    maxm = 26
    INF = float("inf")
    dp = [[INF] * (maxm + 1) for _ in range(N + 1)]
    nxt = [[None] * (maxm + 1) for _ in range(N + 1)]
    for k in range(maxm + 1):
        dp[N][k] = 0
    for i in range(N - 1, -1, -1):
        for k in range(1, maxm + 1):
            for j in range(i + 1, min(N, i + 8) + 1):
                c = min_cap(bs[i:j])
                v = 8 * c - sum(bs[i:j]) + dp[j][k - 1]
                if v < dp[i][k]:
                    dp[i][k] = v
                    nxt[i][k] = j
    i, k = 0, maxm
    caps = []
    asg = [None] * len(blocks)
    while i < N:
        j = nxt[i][k]
        c = min_cap(bs[i:j])
        caps.append(c)
        for bi in range(i, j):
            s = bs[bi]
            pieces = []
            while s > 0:
                u = min(c, s)
                pieces.append((c, u))
                s -= u
            asg[order[bi]] = pieces
        i, k = j, k - 1
    return caps, asg


def _plan(offs):
    """-> (slot_caps [process order], fills): fills[core][slot] =
    (expert, tok_start, n_used) or None."""
    pkey = np.asarray(offs, dtype=np.int64).tobytes()
    if pkey in _PLANS:
        return _PLANS[pkey]
    bounds = np.concatenate([[0], np.asarray(offs, dtype=np.int64)])
    gids = [g for g in range(G_EXP) if bounds[g + 1] > bounds[g]]
    blocks = [int(bounds[g + 1] - bounds[g]) for g in gids]

    caps = _weave(CAPS)
    asg = _fit_global(blocks, caps)
    if asg is None:
        fcaps, asg = _fallback_fit(blocks)
        caps = _weave(fcaps)

    # piece -> (slot position, core).  cap value -> positions (desc order)
    pos_of = {}
    for p, c in enumerate(caps):
        pos_of.setdefault(c, []).append(p)
    cell_cursor = {c: 0 for c in pos_of}  # 0 .. 8*len(positions)-1
    fills = [[None] * len(caps) for _ in range(NCORES)]
    for bi, pieces in enumerate(asg):
        g = gids[bi]
        base = int(bounds[g])
        # stable order: big pieces first
        for cap, used in sorted(pieces, reverse=True):
            if used <= 0:
                continue
            cur = cell_cursor[cap]
            cell_cursor[cap] = cur + 1
            positions = pos_of[cap]
            pos = positions[cur // NCORES]
            core = cur % NCORES
            assert fills[core][pos] is None
            fills[core][pos] = (g, base, used)
            base += used
    _PLANS[pkey] = (caps, fills)
    return caps, fills


def _chunks_of(cap, first=False, last=False):
    """Chunk sizes for one slot."""
    out = []
    rem = cap
    tail = []
    if first and cap > HEAD_CHUNK:
        out.append(HEAD_CHUNK)
        rem -= HEAD_CHUNK
    if last:
        # drain with two small chunks: the penultimate's y transfer
        # clears the queue before the final DMA's init expires
        while rem > 2 * TAIL_CHUNK and len(tail) < 2:
            tail.append(TAIL_CHUNK)
            rem -= TAIL_CHUNK
    while rem > 0:
        n = min(CHUNK, rem)
        out.append(n)
        rem -= n
    return out + tail


# ------------------------------------------------------------ program

def _build_program(caps_key):
    caps = list(caps_key)
    nslots = len(caps)
    cap_tot = sum(caps)
    stream = KTILES * cap_tot

    nc = bacc.Bacc("TRN2", target_bir_lowering=False)
    xt = nc.dram_tensor("xt", [128, stream], BF16, kind="ExternalInput")
    wt = nc.dram_tensor(
        "wt", [nslots, 128, KTILES * DOUT], BF16, kind="ExternalInput")
    y = nc.dram_tensor("y", [128, stream], BF16, kind="ExternalOutput")

    with tile.TileContext(nc) as tc:
        with (
            tc.tile_pool(name="wp", bufs=WBUFS) as wp,
            tc.tile_pool(name="xp", bufs=XBUFS) as xp,
            tc.tile_pool(name="pp", bufs=8, space="PSUM") as pp,
            tc.tile_pool(name="yp", bufs=YBUFS) as yp,
        ):
            # PE p-state warmup: keep the PE busy from ~0.4us so the
            # 3us half-speed ramp is burned on dummies while the first
            # weight/x slices are still in flight.
            dummy = xp.tile([128, 128], BF16, tag="warm", name="warm")
            nc.vector.memset(dummy[:], 0)
            for _ in range(N_WARM):
                pw = pp.tile([128, CHUNK], F32, tag="ps", name="pw")
                nc.tensor.matmul(
                    pw[:1, :128], lhsT=dummy[:, :1], rhs=dummy[:],
                    start=True, stop=True)

            xoff = 0
            for s, cap in enumerate(caps):
                chunk_list = _chunks_of(cap, first=(s == 0),
                                        last=(s == nslots - 1))
                wtile = wp.tile([128, KTILES * DOUT], BF16, tag="w")
                if s == 0:
                    # head: w k-slices on scalar (k0 in four j-quarters
                    # so the first matmul starts at the DMA-latency
                    # floor), x chunk0 k-sliced on sync; chunk0 k-outer.
                    q = DOUT // 4
                    for qi in range(4):
                        nc.scalar.dma_start(
                            out=wtile[:, qi * q:(qi + 1) * q],
                            in_=wt[s][:, qi * q:(qi + 1) * q])
                    for k in range(1, KTILES):
                        nc.scalar.dma_start(
                            out=wtile[:, k * DOUT:(k + 1) * DOUT],
                            in_=wt[s][:, k * DOUT:(k + 1) * DOUT])
                else:
                    # alternate weight queues: gpsimd and scalar each
                    # carry half the slots (y DMAs are latency-tolerant)
                    weng = nc.gpsimd if s % 2 else nc.scalar
                    hw = KTILES * DOUT // 2
                    weng.dma_start(out=wtile[:, :hw], in_=wt[s][:, :hw])
                    weng.dma_start(out=wtile[:, hw:], in_=wt[s][:, hw:])
                for ci, n in enumerate(chunk_list):
                    head = (s == 0 and ci == 0)
                    if head:
                        xtile = xp.tile([128, KTILES * n], BF16, tag="x")
                        for k in range(KTILES):
                            nc.sync.dma_start(
                                out=xtile[:, k * n:(k + 1) * n],
                                in_=xt[:, xoff + k * n:xoff + (k + 1) * n])
                    else:
                        xtile = xp.tile([128, KTILES * n], BF16, tag="x")
                        nc.sync.dma_start(
                            out=xtile[:], in_=xt[:, xoff:xoff + KTILES * n])
                    ytile = yp.tile([128, JTILES * n], BF16, tag="y")
                    if head:
                        # k-outer so each k pass needs only w/x k-slice k
                        pss = [pp.tile([128, CHUNK], F32, tag="ps",
                                       name="ps") for _ in range(JTILES)]
                        for k in range(KTILES):
                            for j in range(JTILES):
                                nc.tensor.matmul(
                                    pss[j][:, :n],
                                    lhsT=wtile[:, k * DOUT + j * 128:
                                               k * DOUT + (j + 1) * 128],
                                    rhs=xtile[:, k * n:(k + 1) * n],
                                    start=(k == 0), stop=(k == KTILES - 1))
                        for j in range(JTILES):
                            nc.vector.tensor_copy(
                                ytile[:, j * n:(j + 1) * n], pss[j][:, :n])
                    elif n <= CHUNK // JTILES:
                        # tiny chunk: all 8 j-slices fit in one PSUM
                        # bank -> 1 tile, 1 trailing copy (no WAR
                        # fan-out against the previous chunk's banks,
                        # and no copy-vs-matmul WAR inside the tile)
                        ps = pp.tile([128, CHUNK], F32, tag="ps")
                        for j in range(JTILES):
                            for k in range(KTILES):
                                nc.tensor.matmul(
                                    ps[:, j * n:(j + 1) * n],
                                    lhsT=wtile[:, k * DOUT + j * 128:
                                               k * DOUT + (j + 1) * 128],
                                    rhs=xtile[:, k * n:(k + 1) * n],
                                    start=(k == 0), stop=(k == KTILES - 1))
                        nc.vector.tensor_copy(
                            ytile[:], ps[:, :JTILES * n])
                    else:
                        for j in range(JTILES):
                            ps = pp.tile([128, CHUNK], F32, tag="ps")
                            for k in range(KTILES):
                                nc.tensor.matmul(
                                    ps[:, :n],
                                    lhsT=wtile[:, k * DOUT + j * 128:
                                               k * DOUT + (j + 1) * 128],
                                    rhs=xtile[:, k * n:(k + 1) * n],
                                    start=(k == 0), stop=(k == KTILES - 1))
                            nc.vector.tensor_copy(
                                ytile[:, j * n:(j + 1) * n], ps[:, :n])
                    nc.scalar.dma_start(
                        out=y[:, xoff:xoff + JTILES * n], in_=ytile[:])
                    xoff += KTILES * n
    nc.compile()
    return nc


# --------------------------------------------------------------- host

def _build_inputs(x, w, caps, fills):
    cap_tot = sum(caps)
    slot_off = np.concatenate([[0], np.cumsum(caps)])
    wt_cache = {}

    def wt_of(g):
        if g not in wt_cache:
            wtg = w[g].T.astype(ml_dtypes.bfloat16)
            wt_cache[g] = np.ascontiguousarray(
                wtg.reshape(KTILES, 128, DOUT).transpose(1, 0, 2)
            ).reshape(128, KTILES * DOUT)
        return wt_cache[g]

    chunks = []
    for s, cap in enumerate(caps):
        o = int(slot_off[s])
        for n in _chunks_of(cap, first=(s == 0), last=(s == len(caps) - 1)):
            chunks.append((o, n))
            o += n

    xts, wts, idxs = [], [], []
    for c in range(NCORES):
        idx = np.full(cap_tot, -1, dtype=np.int64)
        wt_c = np.zeros((len(caps), 128, KTILES * DOUT),
                        dtype=ml_dtypes.bfloat16)
        for s, piece in enumerate(fills[c]):
            if piece is None:
                continue
            g, base, used = piece
            wt_c[s] = wt_of(g)
            o = int(slot_off[s])
            idx[o:o + used] = np.arange(base, base + used)
        xpad = np.zeros((cap_tot, DIN), dtype=ml_dtypes.bfloat16)
        valid = idx >= 0
        xpad[valid] = x[idx[valid]].astype(ml_dtypes.bfloat16)
        xt_c = np.empty((128, KTILES * cap_tot), dtype=ml_dtypes.bfloat16)
        for o, n in chunks:
            xt_c[:, KTILES * o:KTILES * (o + n)] = (
                xpad[o:o + n].reshape(n, KTILES, 128)
                .transpose(2, 1, 0).reshape(128, KTILES * n)
            )
        xts.append(xt_c)
        wts.append(wt_c)
        idxs.append(idx)
    return xts, wts, idxs, cap_tot


def kernel(input, weight, grouped_mm_offs):
    global LAST_EXEC_NS, SPMD_WALL_S
    x = np.ascontiguousarray(np.asarray(input, dtype=np.float32))
    w = np.ascontiguousarray(np.asarray(weight, dtype=np.float32))
    offs = np.asarray(grouped_mm_offs, dtype=np.int32)

    caps, fills = _plan(offs)
    key = tuple(caps)
    if key not in _COMPILED:
        _COMPILED[key] = _build_program(key)
    nc = _COMPILED[key]

    xts, wts, idxs, cap_tot = _build_inputs(x, w, caps, fills)
    in_maps = [{"xt": xts[c], "wt": wts[c]} for c in range(NCORES)]
    t0 = time.time()
    res = run_bass_kernel_spmd(nc, in_maps, core_ids=list(range(NCORES)))
    SPMD_WALL_S = time.time() - t0
    LAST_EXEC_NS = res.exec_time_ns

    slot_off = np.concatenate([[0], np.cumsum(caps)])
    chunks = []
    for s, cap in enumerate(caps):
        o = int(slot_off[s])
        for n in _chunks_of(cap, first=(s == 0), last=(s == len(caps) - 1)):
            chunks.append((o, n))
            o += n

    out = np.zeros((T_TOK, DOUT), dtype=np.float32)
    for c in range(NCORES):
        yb = np.asarray(res.results[c]["y"])
        rows = np.empty((cap_tot, DOUT), dtype=np.float32)
        for o, n in chunks:
            blk = yb[:, KTILES * o:KTILES * (o + n)].reshape(128, JTILES, n)
            rows[o:o + n] = (
                blk.transpose(2, 1, 0).reshape(n, DOUT).astype(np.float32))
        valid = idxs[c] >= 0
        out[idxs[c][valid]] = rows[valid]
    return out
